# revision 39
# baseline (speedup 1.0000x reference)
"""Attention-pooling kernel for TRN2 (8 NeuronCores, data-parallel over batch).

Problem (nn_AttentionPooling3): x [16, 4096, 1024] f32; per head h of 8,
logit[b,h,t] = x[b,t,h*128:(h+1)*128] @ (Q[h] @ key_p[h]) / sqrt(64);
attn = softmax over t; out[b, h*128:(h+1)*128] = sum_t attn * x-slice.

Strategy per core (2 batches/core) — measured-rate balanced fp16 pipeline
(~117-128us vs the 173-175us fp32 baseline):
- DMA: one HWDGE queue (qSP) spreads over all 16 DMA engines and sustains
  ~370 GB/s; the 33.6MB x stream sets a ~93us roofline (device-wide: 8
  cores x 360GB/s = the chip's 2.9TB/s). Everything else overlaps it.
- prod = x*w in fp16 is the central tensor: it feeds the logit reduce AND
  is the PE's moving tensor (y = sum_t e*(x*w); the host divides the tiny
  [B,F] output by w afterward — the w factor cancels exactly, so this
  loses no precision and spares a second converted copy of x).
- ScalarE converts each x unit to fp16 (~0.9us/chunk) so DVE's mul runs
  in 2x mode (~0.57/chunk). The convert is emitted one unit AHEAD of the
  rest (software pipelining) so ScalarE's in-order stream never couples a
  convert behind the previous unit's exp (which waits on DVE).
- Everything elementwise lives on DVE (~5.5us/unit): GP cannot run
  free-axis reduces or InstPool on TRN2 (walrus: "Must be DVE"), and GP
  TTs concurrent with DVE TTs thrash the shared SBUF read ports (measured
  3.5x collapse of both — so GP stays idle). The grouped logit reduce is
  two fp16 halving TTs (2x mode) then a 32-wide fp32 tensor_reduce.
- exp on ScalarE -> e bf16 (fp16 would overflow: logits reach +63; no max
  subtraction needed — softmax is shift-invariant and e^63 fits fp32/
  bf16 range). PE: lhsT = e bf16 x rhs = prod fp16 (mixed 2-byte matmul,
  verified on HW; fp32 PE modes are 2-pass and HAM-throttled ~3x slower),
  fp32 PSUM accumulation over all 32 chunks per batch.
- Normalizer: one ones^T @ e matmul per unit into a [1, nch*8] PSUM row;
  the host finishes s[h] = sum_n (skips an on-chip transpose+recip tail).
- Batch-0's PSUM drain is deferred into the middle of batch 1 and its
  stores go on qAct: both a drain on ScalarE's stream and a store on the
  in-order qSP would stall batch 1 behind batch 0's PSUM stop.
Engine busy (measured, cool chip): DMA ~90 | DVE ~88 | Sc ~65 | PE ~80
(HAM util-capped ~0.47) | GP 0 -> DMA/DVE co-bound plus ~10us start+tail.
"""

import math

import numpy as np

import concourse.bass as bass
import concourse.mybir as mybir
import concourse.tile as tile
from concourse.bass_utils import run_bass_kernel_spmd

B, T, F = 16, 4096, 1024
H, V, KD = 8, 128, 64
NCORES = 8
BL = B // NCORES            # batches per core: 2
NCH = 4                     # 128-row chunks per unit
NCHUNKS = T // 128          # 32
FP32 = mybir.dt.float32
FP16 = mybir.dt.float16
BF16 = mybir.dt.bfloat16


# Work items per batch: (first-128-chunk, n-chunks, mul-engine).
def _items_for(b):
    # All elementwise work on DVE (see module docstring for why GP sits
    # idle). Finer units at the start of batch 0 prime the pipeline; a
    # fine tail on batch 1 shortens the last unit's serial chain.
    if b == 0:
        return [
            (0, 1, "ve"), (1, 1, "ve"), (2, 2, "ve"),
            (4, 4, "ve"), (8, 4, "ve"), (12, 4, "ve"), (16, 4, "ve"),
            (20, 4, "ve"), (24, 4, "ve"), (28, 4, "ve"),
        ]
    return [
        (0, 4, "ve"), (4, 4, "ve"), (8, 4, "ve"), (12, 4, "ve"),
        (16, 4, "ve"), (20, 4, "ve"), (24, 4, "ve"),
        (28, 2, "ve"), (30, 1, "ve"), (31, 1, "ve"),
    ]


def _build_nc():
    nc = bass.Bass()
    x_d = nc.declare_dram_parameter("x", [BL, T, F], FP32, isOutput=False)
    wb_d = nc.declare_dram_parameter("wb", [128, F], FP32, isOutput=False)
    wh_d = nc.declare_dram_parameter("wh", [128, F], FP16, isOutput=False)
    y_d = nc.declare_dram_parameter("y", [BL, H, F], FP32, isOutput=True)
    # Raw per-(n,h) normalizer sums; the host finishes s[h] = sum_n and
    # divides (cheaper than an on-chip transpose + reciprocal on the tail).
    s_d = nc.declare_dram_parameter("s", [BL, 1, NCH * H], FP32, isOutput=True)

    with tile.TileContext(nc) as tc:
        with (
            tc.tile_pool(name="const", bufs=1) as const_pool,
            tc.tile_pool(name="xin", bufs=5) as xpool,
            tc.tile_pool(name="xh", bufs=4) as xhpool,
            tc.tile_pool(name="prod", bufs=5) as ppool,
            tc.tile_pool(name="half", bufs=3) as hpool,
            tc.tile_pool(name="quar", bufs=3) as qpool,
            tc.tile_pool(name="small", bufs=4) as small,
            tc.tile_pool(name="acc", bufs=1, space="PSUM") as psum_pool,
        ):
            # Weight rows load once on the Scalar HWDGE queue so they don't
            # delay unit 0's x load on the Sync queue.
            any_gp = any(
                eng == "gp" for b in range(BL) for _, _, eng in _items_for(b)
            )
            if any_gp:
                wb_sb = const_pool.tile([128, F], FP32)
                nc.scalar.dma_start(out=wb_sb, in_=wb_d[:, :])
            wh_sb = const_pool.tile([128, F], FP16)
            nc.scalar.dma_start(out=wh_sb, in_=wh_d[:, :])
            ones_sb = const_pool.tile([128, 1], BF16)
            nc.vector.memset(ones_sb, 1.0)
            # Touch Exp once at t=0 so the activation table loads during the
            # DMA ramp instead of on the first real exp's critical path.
            warm_sb = const_pool.tile([1, 1], FP32)
            nc.scalar.activation(
                out=warm_sb,
                in_=ones_sb[0:1, :].bitcast(BF16),
                func=mybir.ActivationFunctionType.Exp,
            )

            def emit_head(b, ch0, nch):
                """DMA + convert for one unit. Emitted one unit AHEAD of the
                rest so ScalarE's in-order stream never couples a convert
                behind the previous unit's exp (which waits on DVE)."""
                xt = xpool.tile([128, NCH, F], FP32, name="xt")
                xt_v = xt[:, :nch, :]
                nc.sync.dma_start(
                    out=xt_v,
                    in_=x_d[
                        b, ch0 * 128 : (ch0 + nch) * 128, :
                    ].rearrange("(n p) f -> p n f", p=128),
                )
                xh = xhpool.tile([128, NCH, F], FP16, name="xh")
                xh_v = xh[:, :nch, :]
                nc.scalar.activation(
                    out=xh_v,
                    in_=xt_v,
                    func=mybir.ActivationFunctionType.Copy,
                )
                return xh

            def emit_body(b, ch0, nch, eng, xh, pooled_ps, s_ps, first, last):
                xh_v = xh[:, :nch, :]
                prod = ppool.tile([128, NCH, F], FP16, name="prod")
                prod_v = prod[:, :nch, :]
                wh_bc = bass.AP(
                    tensor=wh_sb.tensor,
                    offset=wh_sb.offset,
                    ap=[wh_sb.ap[0], [0, nch], wh_sb.ap[1]],
                )
                nc.vector.tensor_mul(prod_v, xh_v, wh_bc)
                # Grouped logit reduce over v=128, DVE-only: two fp16
                # halving TTs in 2x mode, then a 32-wide fp32 reduce. fp16
                # partial sums of x*w terms add <=1e-3 to logits
                # (validated off-line against the 2e-2 gate).
                prod_hv = prod_v.rearrange("p n (h v) -> p n h v", v=V)
                half_t = hpool.tile([128, NCH, H, V // 2], FP16, name="half_t")
                quar_t = qpool.tile([128, NCH, H, V // 4], FP16, name="quar_t")
                with nc.allow_low_precision(
                    reason="fp16 pair sums of x*w; logits stay fp32 after"
                ):
                    nc.vector.tensor_add(
                        half_t[:, :nch, :, :],
                        prod_hv[:, :, :, 0 : V // 2],
                        prod_hv[:, :, :, V // 2 : V],
                    )
                    nc.vector.tensor_add(
                        quar_t[:, :nch, :, :],
                        half_t[:, :nch, :, 0 : V // 4],
                        half_t[:, :nch, :, V // 4 : V // 2],
                    )
                logits_u = small.tile([128, NCH, H], FP32, name="logits_u")
                nc.vector.tensor_reduce(
                    logits_u[:, :nch, :],
                    quar_t[:, :nch, :, :],
                    axis=mybir.AxisListType.X,
                    op=mybir.AluOpType.add,
                )
                e_u = small.tile([128, NCH, H], BF16, name="e_u")
                nc.scalar.activation(
                    out=e_u[:, :nch, :],
                    in_=logits_u[:, :nch, :],
                    func=mybir.ActivationFunctionType.Exp,
                )
                # Group matmuls by PSUM bank (all low halves, then all high
                # halves): per-MM bank alternation causes HAM re-throttle
                # and blocks MM pipelining.
                for half in range(2):
                    lo, hi = half * 512, half * 512 + 512
                    for n in range(nch):
                        ch = ch0 + n
                        nc.tensor.matmul(
                            pooled_ps[:, lo:hi],
                            e_u[:, n, :],
                            prod[:, n, lo:hi],
                            start=ch == 0,
                            stop=ch == NCHUNKS - 1,
                        )
                # One normalizer matmul per unit: ones^T @ e gives the
                # per-(n,h) partial sums as a [1, nch*8] PSUM row; the host
                # finishes the n-sum (units with nch<4 just fold their
                # chunks into the low n slots — still a complete sum).
                nc.tensor.matmul(
                    s_ps[:, 0 : nch * H],
                    ones_sb,
                    e_u[:, :nch, :],
                    start=first,
                    stop=last,
                )

            # Flatten both batches into one software-pipelined stream:
            # head(i+1) is emitted before body(i).
            sched = []
            for b in range(BL):
                pooled_ps = psum_pool.tile([H, F], FP32, name=f"pooled{b}")
                s_ps = psum_pool.tile([1, NCH * H], FP32, name=f"s{b}")
                items = _items_for(b)
                for it_idx, (ch0, nch, eng) in enumerate(items):
                    sched.append(
                        (b, ch0, nch, eng, pooled_ps, s_ps,
                         it_idx == 0, it_idx == len(items) - 1)
                    )
            def emit_drain(b, pooled_ps, s_ps):
                y_sb = small.tile([H, F], FP32, name="y_sb")
                nc.scalar.activation(
                    out=y_sb,
                    in_=pooled_ps,
                    func=mybir.ActivationFunctionType.Copy,
                )
                s_sb = small.tile([1, NCH * H], FP32, name="s_sb")
                nc.vector.tensor_copy(s_sb, s_ps)
                # qAct, not qSP: HWDGE queues are in-order, and a y store
                # on qSP would stall the next batch's x loads behind the
                # PSUM drain.
                nc.scalar.dma_start(out=y_d[b], in_=y_sb)
                nc.scalar.dma_start(out=s_d[b], in_=s_sb)

            heads = [None] * len(sched)
            heads[0] = emit_head(*sched[0][:3])
            pending_drain = None
            for i in range(len(sched)):
                if i + 1 < len(sched):
                    heads[i + 1] = emit_head(*sched[i + 1][:3])
                b, ch0, nch, eng, pooled_ps, s_ps, first, last = sched[i]
                emit_body(
                    b, ch0, nch, eng, heads[i], pooled_ps, s_ps, first, last
                )
                if last:
                    if b == BL - 1:
                        emit_drain(b, pooled_ps, s_ps)
                    else:
                        # Defer this batch's PSUM drain: emitted mid-way
                        # through the next batch so Scalar's in-order stream
                        # doesn't couple the next batch's converts behind
                        # this batch's PSUM stop (which waits on the PE).
                        pending_drain = (b, pooled_ps, s_ps)
                elif pending_drain is not None and ch0 >= 12:
                    emit_drain(*pending_drain)
                    pending_drain = None
    return nc


def _split_multiwaits(nc, limit=1):
    """This container's walrus accepts at most `limit` sync-wait commands per
    instruction ("Too many sync wait commands" otherwise). Tile attaches up to
    ~12. Move excess waits onto preceding same-engine NoOps — semantics are
    unchanged (waits are AND conditions that block the engine either way)."""
    for fn in nc.m.functions:
        for blk in fn.blocks:
            new = []
            for inst in blk.instructions:
                si = getattr(inst, "sync_info", None)
                ow = list(si.on_wait) if si is not None and si.on_wait else []
                if len(ow) > limit:
                    extra, keep = ow[:-limit], ow[-limit:]
                    for i in range(0, len(extra), limit):
                        new.append(
                            mybir.InstNoOp(
                                name=f"{inst.name}-wsplit{i}",
                                engine=inst.engine,
                                ins=[],
                                outs=[],
                                sync_info=mybir.SyncInfo(
                                    on_wait=extra[i : i + limit], on_update=[]
                                ),
                            )
                        )
                    inst.sync_info = mybir.SyncInfo(
                        on_wait=keep, on_update=si.on_update
                    )
                new.append(inst)
            blk.instructions = new


_NC = None


def _get_nc():
    global _NC
    if _NC is None:
        _NC = _build_nc()
        _split_multiwaits(_NC)
    return _NC


def _fold_weights(Q, key_p):
    w = np.einsum(
        "hvk,hk->hv", np.asarray(Q, np.float32), np.asarray(key_p, np.float32)[:, :, 0]
    ) / np.float32(math.sqrt(KD))
    return w.reshape(H * V).astype(np.float32)


def _run(x, Q, key_p, trace=False, tmpdir=None):
    x = np.ascontiguousarray(np.asarray(x, np.float32))
    w_flat = _fold_weights(Q, key_p)
    wb = np.tile(w_flat.reshape(1, H * V), (128, 1))
    wh = wb.astype(np.float16)
    nc = _get_nc()
    in_maps = [
        {"x": x[c * BL : (c + 1) * BL], "wb": wb, "wh": wh}
        for c in range(NCORES)
    ]
    res = run_bass_kernel_spmd(
        nc, in_maps, list(range(NCORES)), trace=trace, tmpdir=tmpdir
    )
    # Kernel returns raw sum_t e*(x*w) plus per-(n,h) normalizer partials;
    # the host finishes s, then divides out s and w (the fp16 rounding of
    # w cancels up to ~2.5e-4 between the logits and the pooled product).
    y = np.empty((B, F), np.float32)
    for c in range(NCORES):
        yc = res.results[c]["y"]  # [BL, H, F]
        sc = res.results[c]["s"]  # [BL, 1, NCH*H]
        for b in range(BL):
            s8 = sc[b, 0].reshape(NCH, H).sum(0, dtype=np.float32)
            for h in range(H):
                sl = slice(h * V, (h + 1) * V)
                y[c * BL + b, sl] = yc[b, h, sl] / (s8[h] * w_flat[sl])
    return y, res


def kernel(**inputs):
    y, _ = _run(inputs["x"], inputs["Q"], inputs["key_p"])
    return y
